# revision 23
# baseline (speedup 1.0000x reference)
"""Trainium2 Bass kernel for nn_DeepNNDendroMatrix.

Math (reference):
    cols = path_mat[:, node_idx]                       # (E, B) in {0,1}
    layer(h, root, delta): relu(h @ root + sum_e cols[e,b] * (h @ W_e))
        where W_e[i,o] = delta[o,i,e]
    out = squeeze(layer2(layer1(x)))

Factorization used here (avoids the (B,in,out) intermediate):
    h1[b,o] = relu( (x@root0)[b,o] + sum_e colsT[b,e] * (x @ W_e)[b,o] )
    out[b]  = relu( sum_{e'} colsAug[b,e'] * (h1 @ W2)[b,e'] )
        W2 = [delta1[0] | root1 | 0]   (H, 130),  colsAug = [colsT | 1 | 0]

Distribution: data-parallel over batch. 8 cores x 256 samples. Each core
streams the full (rearranged, bf16) delta0 once from HBM (33.5 MB), keeps
x^T resident in SBUF as the matmul stationary operand, accumulates the
per-edge scaled matmul outputs on the vector engine with fused
scalar_tensor_tensor (acc = psum_e * colsT[:,e] + acc).
"""

import numpy as np
import ml_dtypes

import concourse.bass as bass
import concourse.mybir as mybir
from concourse.tile import TileContext
from concourse.bass_utils import run_bass_kernel_spmd

# ---------------------------------------------------------------------------
# Workaround: this walrus build allows only ONE sync wait per CTRL (Drain)
# instruction; TileContext's tail drain aggregates one wait per live
# semaphore onto a single Drain. Split them across multiple Drains.
import bass_rust
import concourse.tile as _tile_mod
from concourse.vector_clock import ScopedClock as _ScopedClock

_MAX_WAITS_PER_INST = 1


def _split_drain_and_barrier(self, tick_clock, wait_clock):
    nc = self.nc
    drain_inst = nc.sync.drain()
    wait_clock.add_sem_waits(
        drain_inst.ins, _ScopedClock({None: tick_clock.global_clock})
    )
    si = drain_inst.ins.sync_info
    waits = list(si.on_wait) if si is not None else []
    if len(waits) > _MAX_WAITS_PER_INST:
        si.on_wait = waits[:_MAX_WAITS_PER_INST]
        rest = waits[_MAX_WAITS_PER_INST:]
        for i in range(0, len(rest), _MAX_WAITS_PER_INST):
            extra = nc.sync.drain()
            chunk = rest[i : i + _MAX_WAITS_PER_INST]
            esi = extra.ins.sync_info
            if esi is None:
                extra.ins.sync_info = bass_rust.SyncInfo(on_wait=chunk, on_update=[])
            else:
                esi.on_wait = list(esi.on_wait) + chunk
    nc.all_engine_barrier()
    assert self.sems is not None
    popped = nc._tile_sem_poison_stack.pop()
    assert popped is self._sem_poison
    nc.clear_and_free_semaphores(list(self.sems.allocated().values()))
    nc.all_engine_barrier()


_tile_mod.TileContext._drain_and_barrier = _split_drain_and_barrier


_COALESCE_OK = {"Ldweights", "NoOp", "TensorCopy", "Memset", "TensorScalarPtr",
                "Matmult", "Activation", "TensorScalar"}


import os as _os2

_WAIT_CAP_DEFAULT = int(_os2.environ.get("KW_WAIT_CAP", "1"))


def _legalize_wait_counts(nc, max_waits=None):
    """Split any instruction carrying more than `max_waits` sync waits.

    Moving a wait onto an earlier instruction of the same engine is always
    safe (the engine just blocks earlier), so first try to coalesce excess
    waits onto the immediately-preceding same-engine instruction if it has
    spare wait slots; otherwise insert a NoOp carrying the wait."""
    if max_waits is None:
        max_waits = _WAIT_CAP_DEFAULT
    n_nops = 0
    for f in nc.m.functions:
        for bb in f.blocks:
            out = []
            for inst in bb.instructions:
                si = inst.sync_info
                waits = list(si.on_wait) if si is not None else []
                if len(waits) > max_waits:
                    si.on_wait = waits[:max_waits]
                    rest = waits[max_waits:]
                    # try to place excess on the immediately-preceding
                    # same-engine instruction (moving a wait earlier on the
                    # same engine is always safe, as long as that instruction
                    # does not itself update the awaited semaphore)
                    if out:
                        prev = out[-1]
                        if prev.engine == inst.engine and prev.opcode in _COALESCE_OK:
                            psi = prev.sync_info
                            pw = list(psi.on_wait) if psi is not None else []
                            upd_ids = {
                                u.id
                                for u in (psi.on_update if psi is not None else [])
                            }
                            while (
                                rest
                                and len(pw) < max_waits
                                and rest[0].id not in upd_ids
                            ):
                                pw.append(rest.pop(0))
                            if pw:
                                if psi is None:
                                    prev.sync_info = bass_rust.SyncInfo(
                                        on_wait=pw, on_update=[]
                                    )
                                else:
                                    psi.on_wait = pw
                    for i in range(0, len(rest), max_waits):
                        nop = bass_rust.InstNoOp(
                            name=f"{inst.name}-ws{i}", engine=inst.engine,
                            ins=[], outs=[],
                        )
                        nop.sync_info = bass_rust.SyncInfo(
                            on_wait=rest[i : i + max_waits], on_update=[]
                        )
                        out.append(nop)
                        n_nops += 1
                out.append(inst)
            bb.instructions = out
    return n_nops
# ---------------------------------------------------------------------------

# ---------------------------------------------------------------------------
# Persistent NEFF cache: walrus compilation of this kernel takes minutes and
# bass2jax recompiles per process. Cache the compiled NEFF on disk keyed by
# the BIR sha256 so repeat processes skip the compile.
import hashlib as _hashlib
import os as _os
import shutil as _shutil

import concourse.bass2jax as _bass2jax
import concourse.bass_utils as _bass_utils_mod

_NEFF_CACHE_DIR = _os.path.expanduser("~/.cache/bass_neff")
_orig_compile_bir_kernel = _bass_utils_mod.compile_bir_kernel


def _cached_compile_bir_kernel(bir_json, tmpdir, neff_name="file.neff"):
    try:
        raw = bir_json if isinstance(bir_json, bytes) else bir_json.encode()
        # BIR debug info embeds this file's absolute path, which varies with
        # the directory kernel.py is run from - normalize it out of the key.
        norm = raw.replace(_os.path.abspath(__file__).encode(), b"KERNEL_PY")
        key = _hashlib.sha256(norm).hexdigest()
        cpath = _os.path.join(_NEFF_CACHE_DIR, f"{key}_{neff_name}")
        if _os.path.exists(cpath):
            dst = _os.path.join(tmpdir, "sg00")
            _os.makedirs(dst, exist_ok=True)
            dst_neff = _os.path.join(dst, neff_name)
            _shutil.copy(cpath, dst_neff)
            return dst_neff
    except Exception:
        return _orig_compile_bir_kernel(bir_json, tmpdir, neff_name)
    neff_path = _orig_compile_bir_kernel(bir_json, tmpdir, neff_name)
    try:
        _os.makedirs(_NEFF_CACHE_DIR, exist_ok=True)
        tmp = cpath + ".tmp"
        _shutil.copy(neff_path, tmp)
        _os.replace(tmp, cpath)
    except Exception:
        pass
    return neff_path


_bass2jax.compile_bir_kernel = _cached_compile_bir_kernel
_bass_utils_mod.compile_bir_kernel = _cached_compile_bir_kernel
# ---------------------------------------------------------------------------

NCORES = 8
B, F, H, O, E, N_NODES = 2048, 512, 256, 1, 128, 4096
BL = B // NCORES          # samples per core = 256
NBT = BL // 128           # b-tiles per core = 2
EP = E // 2               # e-pairs = 64
KI = F // 128             # contraction chunks over input features = 4
W2N = 130                 # [delta1 | root1 | zero-pad] free dim (even)

F32 = mybir.dt.float32
BF16 = mybir.dt.bfloat16
MULT = mybir.AluOpType.mult
ADD = mybir.AluOpType.add
RELU = mybir.ActivationFunctionType.Relu
COPY = mybir.ActivationFunctionType.Copy

_CACHE = {}


def _build_nc():
    from concourse.masks import make_identity

    nc = bass.Bass()
    xt_d = nc.dram_tensor("xt", (F, BL), BF16, kind="ExternalInput")
    dl_d = nc.dram_tensor("dl", (EP, F, 2 * H), BF16, kind="ExternalInput")
    r0_d = nc.dram_tensor("r0", (F, H), BF16, kind="ExternalInput")
    cols_d = nc.dram_tensor("cols", (BL, W2N), F32, kind="ExternalInput")
    w2_d = nc.dram_tensor("w2", (H, W2N), F32, kind="ExternalInput")
    out_d = nc.dram_tensor("out", (BL, 1), F32, kind="ExternalOutput")

    with TileContext(nc) as tc:
        with (
            tc.tile_pool(name="const", bufs=1) as cpool,
            tc.tile_pool(name="acc", bufs=NBT) as apool,
            tc.tile_pool(name="dl", bufs=6) as dpool,
            tc.tile_pool(name="psum", bufs=6, space="PSUM") as ppool,
            tc.tile_pool(name="psum_s", bufs=2, space="PSUM") as pspool,
            tc.tile_pool(name="stage", bufs=6) as spool,
            tc.tile_pool(name="sc", bufs=4) as scpool,
            tc.tile_pool(name="misc", bufs=8) as mpool,
        ):
            # --- resident loads -------------------------------------------
            xt_sb = cpool.tile([128, KI * BL], BF16, tag="xt")
            nc.sync.dma_start(
                xt_sb[:].rearrange("p (k b) -> p k b", k=KI),
                xt_d[:].rearrange("(k p) b -> p k b", p=128),
            )
            r0_sb = cpool.tile([128, KI * H], BF16, tag="r0")
            nc.sync.dma_start(
                r0_sb[:].rearrange("p (k o) -> p k o", k=KI),
                r0_d[:].rearrange("(k p) o -> p k o", p=128),
            )
            cols_sb = cpool.tile([128, NBT * W2N], F32, tag="cols")
            nc.sync.dma_start(
                cols_sb[:].rearrange("p (t n) -> p t n", t=NBT),
                cols_d[:].rearrange("(t p) n -> p t n", p=128),
            )
            w2_sb = cpool.tile([128, (H // 128) * W2N], F32, tag="w2")
            nc.sync.dma_start(
                w2_sb[:].rearrange("p (k n) -> p k n", k=H // 128),
                w2_d[:].rearrange("(k p) n -> p k n", p=128),
            )
            ident = cpool.tile([128, 128], F32, tag="ident")
            make_identity(nc, ident[:])

            def x_lhsT(k, bt):
                # stationary operand: x^T chunk [128 (i), 128 (b)]
                return xt_sb[:, k * BL + bt * 128 : k * BL + bt * 128 + 128]

            # --- acc init: acc[bt] = x @ root0 ----------------------------
            accs = []
            accg = []
            for bt in range(NBT):
                ps = pspool.tile([128, H], F32, tag="ps_s")
                for k in range(KI):
                    nc.tensor.matmul(
                        ps[:],
                        x_lhsT(k, bt),
                        r0_sb[:, k * H : (k + 1) * H],
                        start=(k == 0),
                        stop=(k == KI - 1),
                    )
                acc = apool.tile([128, H], F32, tag="acc")
                nc.scalar.activation(acc[:], ps[:], COPY)
                accs.append(acc)
                # second accumulator for the GPSIMD-routed edge slots
                ag = apool.tile([128, H], F32, tag="accg")
                nc.gpsimd.memset(ag[:], 0.0)
                accg.append(ag)

            # --- stage 1: stream delta, accumulate scaled matmuls ---------
            # PE: ps = x @ [W_{2ep} | W_{2ep+1}] ; ACT: evacuate PSUM->SBUF;
            # DVE: acc = stage_half * colsT[:, e] + acc (all-SBUF fused op)
            for ep in range(EP):
                dlt = dpool.tile([128, KI * 2 * H], BF16, tag="dl")
                nc.sync.dma_start(
                    dlt[:].rearrange("p (k n) -> p k n", k=KI),
                    dl_d[ep].rearrange("(k p) n -> p k n", p=128),
                )
                for bt in range(NBT):
                    ps = ppool.tile([128, 2 * H], F32, tag="ps")
                    for k in range(KI):
                        nc.tensor.matmul(
                            ps[:],
                            x_lhsT(k, bt),
                            dlt[:, k * 2 * H : (k + 1) * 2 * H],
                            start=(k == 0),
                            stop=(k == KI - 1),
                        )
                    stage = spool.tile([128, 2 * H], F32, tag="stage")
                    nc.scalar.activation(stage[:], ps[:], COPY)
                    for half in range(2):
                        e = 2 * ep + half
                        half_ap = stage[:, half * H : (half + 1) * H]
                        col_ap = cols_sb[:, bt * W2N + e : bt * W2N + e + 1]
                        if e % 3 == 2:
                            # route every 3rd edge slot via a cheap DVE scaled
                            # copy (tensor_scalar runs in 2x mode, under the
                            # DRAIN threshold) + a GPSIMD add into a second
                            # accumulator, shortening the serial DVE STT chain
                            sc = scpool.tile([128, H], F32, tag="sc")
                            nc.vector.tensor_scalar(
                                sc[:], half_ap, col_ap, None, MULT
                            )
                            nc.gpsimd.tensor_add(accg[bt][:], sc[:], accg[bt][:])
                        else:
                            nc.vector.scalar_tensor_tensor(
                                out=accs[bt][:],
                                in0=half_ap,
                                scalar=col_ap,
                                in1=accs[bt][:],
                                op0=MULT,
                                op1=ADD,
                            )

            # --- layer 2 ---------------------------------------------------
            for bt in range(NBT):
                # merge the GPSIMD accumulator, then relu
                nc.vector.tensor_add(accs[bt][:], accg[bt][:], accs[bt][:])
                h1 = mpool.tile([128, H], F32, tag="h1")
                nc.scalar.activation(h1[:], accs[bt][:], RELU)
                h1t = mpool.tile([128, H], F32, tag="h1t")
                for k in range(H // 128):
                    pst = pspool.tile([128, 128], F32, tag="ps_s")
                    nc.tensor.transpose(pst[:], h1[:, k * 128 : (k + 1) * 128], ident[:])
                    nc.scalar.activation(h1t[:, k * 128 : (k + 1) * 128], pst[:], COPY)
                ps2 = pspool.tile([128, W2N], F32, tag="ps_s")
                for k in range(H // 128):
                    nc.tensor.matmul(
                        ps2[:],
                        h1t[:, k * 128 : (k + 1) * 128],
                        w2_sb[:, k * W2N : (k + 1) * W2N],
                        start=(k == 0),
                        stop=(k == H // 128 - 1),
                    )
                junk = mpool.tile([128, W2N], F32, tag="junk")
                res = mpool.tile([128, 1], F32, tag="res")
                nc.vector.scalar_tensor_tensor(
                    out=junk[:],
                    in0=ps2[:],
                    scalar=1.0,
                    in1=cols_sb[:, bt * W2N : (bt + 1) * W2N],
                    op0=MULT,
                    op1=MULT,
                    accum_out=res[:],
                )
                resr = mpool.tile([128, 1], F32, tag="resr")
                nc.scalar.activation(resr[:], res[:], RELU)
                nc.sync.dma_start(
                    out_d[:].rearrange("(t p) o -> t p o", p=128)[bt], resr[:]
                )
    _legalize_wait_counts(nc)
    return nc


def _prep_inputs(x, node_idx, path_mat, root0, root1, delta0, delta1):
    bf16 = ml_dtypes.bfloat16
    x = np.asarray(x, np.float32)
    path_mat = np.asarray(path_mat, np.float32)
    root0 = np.asarray(root0, np.float32)
    root1 = np.asarray(root1, np.float32)
    delta0 = np.asarray(delta0, np.float32)
    delta1 = np.asarray(delta1, np.float32)
    colsT = path_mat.T[np.asarray(node_idx, dtype=np.int64)].astype(np.float32)
    colsA = np.concatenate(
        [colsT, np.ones((B, 1), np.float32), np.zeros((B, 1), np.float32)], axis=1
    )  # (B, 130)
    # delta0 (H, F, E) -> (E, F, H) -> pair e's along the free axis
    dt_ = np.ascontiguousarray(delta0.transpose(2, 1, 0))
    dl = np.ascontiguousarray(
        dt_.reshape(EP, 2, F, H).transpose(0, 2, 1, 3)
    ).reshape(EP, F, 2 * H)
    dl16 = dl.astype(bf16)
    xT16 = np.ascontiguousarray(x.T).astype(bf16)  # (F, B)
    r016 = root0.astype(bf16)
    w2 = np.concatenate(
        [delta1[0], root1, np.zeros((H, 1), np.float32)], axis=1
    ).astype(np.float32)  # (H, 130)

    in_maps = []
    for c in range(NCORES):
        sl = slice(c * BL, (c + 1) * BL)
        in_maps.append(
            {
                "xt": np.ascontiguousarray(xT16[:, sl]),
                "dl": dl16,
                "r0": r016,
                "cols": np.ascontiguousarray(colsA[sl]),
                "w2": w2,
            }
        )
    return in_maps


def _run(inputs, trace=False, **kw):
    if "nc" not in _CACHE:
        _CACHE["nc"] = _build_nc()
    nc = _CACHE["nc"]
    in_maps = _prep_inputs(**inputs)
    res = run_bass_kernel_spmd(
        nc, in_maps, core_ids=list(range(NCORES)), trace=trace, **kw
    )
    out = np.concatenate([r["out"][:, 0] for r in res.results]).astype(np.float32)
    return out, res


def kernel(**inputs) -> np.ndarray:
    out, _ = _run(inputs)
    return out


# revision 34
# speedup vs baseline: 1.0072x; 1.0072x over previous
"""Trainium2 Bass kernel for nn_DeepNNDendroMatrix.

Math (reference):
    cols = path_mat[:, node_idx]                       # (E, B) in {0,1}
    layer(h, root, delta): relu(h @ root + sum_e cols[e,b] * (h @ W_e))
        where W_e[i,o] = delta[o,i,e]
    out = squeeze(layer2(layer1(x)))

Factorization used here (avoids the (B,in,out) intermediate):
    h1[b,o] = relu( (x@root0)[b,o] + sum_e colsT[b,e] * (x @ W_e)[b,o] )
    out[b]  = relu( sum_h h1[b,h] * wt[b,h] )
        wt[b,h] = root1[h] + sum_e colsT[b,e]*delta1[0,h,e]  (computed up
        front on the PE as colsE^T @ delta1[0]^T + ones @ root1^T)

Distribution: data-parallel over batch. 8 cores x 256 samples. Each core
streams the full (rearranged, bf16) delta0 once from HBM (33.5 MB), keeps
x^T resident in SBUF as the matmul stationary operand, accumulates the
per-edge scaled matmul outputs on the vector engine with fused
scalar_tensor_tensor (acc = psum_e * colsT[:,e] + acc).
"""

import numpy as np
import ml_dtypes

import concourse.bass as bass
import concourse.mybir as mybir
from concourse.tile import TileContext
from concourse.bass_utils import run_bass_kernel_spmd

# ---------------------------------------------------------------------------
# Workaround: this walrus build allows only ONE sync wait per CTRL (Drain)
# instruction; TileContext's tail drain aggregates one wait per live
# semaphore onto a single Drain. Split them across multiple Drains.
import bass_rust
import concourse.tile as _tile_mod
from concourse.vector_clock import ScopedClock as _ScopedClock

_MAX_WAITS_PER_INST = 1


def _split_drain_and_barrier(self, tick_clock, wait_clock):
    nc = self.nc
    drain_inst = nc.sync.drain()
    wait_clock.add_sem_waits(
        drain_inst.ins, _ScopedClock({None: tick_clock.global_clock})
    )
    si = drain_inst.ins.sync_info
    waits = list(si.on_wait) if si is not None else []
    if len(waits) > _MAX_WAITS_PER_INST:
        si.on_wait = waits[:_MAX_WAITS_PER_INST]
        rest = waits[_MAX_WAITS_PER_INST:]
        for i in range(0, len(rest), _MAX_WAITS_PER_INST):
            extra = nc.sync.drain()
            chunk = rest[i : i + _MAX_WAITS_PER_INST]
            esi = extra.ins.sync_info
            if esi is None:
                extra.ins.sync_info = bass_rust.SyncInfo(on_wait=chunk, on_update=[])
            else:
                esi.on_wait = list(esi.on_wait) + chunk
    nc.all_engine_barrier()
    assert self.sems is not None
    popped = nc._tile_sem_poison_stack.pop()
    assert popped is self._sem_poison
    nc.clear_and_free_semaphores(list(self.sems.allocated().values()))
    nc.all_engine_barrier()


_tile_mod.TileContext._drain_and_barrier = _split_drain_and_barrier


_COALESCE_OK = {"Ldweights", "NoOp", "TensorCopy", "Memset", "TensorScalarPtr",
                "Matmult", "Activation", "TensorScalar"}


import os as _os2

_WAIT_CAP_DEFAULT = int(_os2.environ.get("KW_WAIT_CAP", "1"))


def _legalize_wait_counts(nc, max_waits=None):
    """Split any instruction carrying more than `max_waits` sync waits.

    Moving a wait onto an earlier instruction of the same engine is always
    safe (the engine just blocks earlier), so first try to coalesce excess
    waits onto the immediately-preceding same-engine instruction if it has
    spare wait slots; otherwise insert a NoOp carrying the wait."""
    if max_waits is None:
        max_waits = _WAIT_CAP_DEFAULT
    n_nops = 0
    for f in nc.m.functions:
        for bb in f.blocks:
            out = []
            for inst in bb.instructions:
                si = inst.sync_info
                waits = list(si.on_wait) if si is not None else []
                if len(waits) > max_waits:
                    si.on_wait = waits[:max_waits]
                    rest = waits[max_waits:]
                    # try to place excess on the immediately-preceding
                    # same-engine instruction (moving a wait earlier on the
                    # same engine is always safe, as long as that instruction
                    # does not itself update the awaited semaphore)
                    if out:
                        prev = out[-1]
                        if prev.engine == inst.engine and prev.opcode in _COALESCE_OK:
                            psi = prev.sync_info
                            pw = list(psi.on_wait) if psi is not None else []
                            upd_ids = {
                                u.id
                                for u in (psi.on_update if psi is not None else [])
                            }
                            while (
                                rest
                                and len(pw) < max_waits
                                and rest[0].id not in upd_ids
                            ):
                                pw.append(rest.pop(0))
                            if pw:
                                if psi is None:
                                    prev.sync_info = bass_rust.SyncInfo(
                                        on_wait=pw, on_update=[]
                                    )
                                else:
                                    psi.on_wait = pw
                    for i in range(0, len(rest), max_waits):
                        nop = bass_rust.InstNoOp(
                            name=f"{inst.name}-ws{i}", engine=inst.engine,
                            ins=[], outs=[],
                        )
                        nop.sync_info = bass_rust.SyncInfo(
                            on_wait=rest[i : i + max_waits], on_update=[]
                        )
                        out.append(nop)
                        n_nops += 1
                out.append(inst)
            bb.instructions = out
    return n_nops
# ---------------------------------------------------------------------------

# ---------------------------------------------------------------------------
# Persistent NEFF cache: walrus compilation of this kernel takes minutes and
# bass2jax recompiles per process. Cache the compiled NEFF on disk keyed by
# the BIR sha256 so repeat processes skip the compile.
import hashlib as _hashlib
import os as _os
import shutil as _shutil

import concourse.bass2jax as _bass2jax
import concourse.bass_utils as _bass_utils_mod

_NEFF_CACHE_DIR = _os.path.expanduser("~/.cache/bass_neff")
_orig_compile_bir_kernel = _bass_utils_mod.compile_bir_kernel


def _cached_compile_bir_kernel(bir_json, tmpdir, neff_name="file.neff"):
    try:
        raw = bir_json if isinstance(bir_json, bytes) else bir_json.encode()
        # BIR debug info embeds this file's absolute path, which varies with
        # the directory kernel.py is run from - normalize it out of the key.
        norm = raw.replace(_os.path.abspath(__file__).encode(), b"KERNEL_PY")
        key = _hashlib.sha256(norm).hexdigest()
        cpath = _os.path.join(_NEFF_CACHE_DIR, f"{key}_{neff_name}")
        if _os.path.exists(cpath):
            dst = _os.path.join(tmpdir, "sg00")
            _os.makedirs(dst, exist_ok=True)
            dst_neff = _os.path.join(dst, neff_name)
            _shutil.copy(cpath, dst_neff)
            return dst_neff
    except Exception:
        return _orig_compile_bir_kernel(bir_json, tmpdir, neff_name)
    neff_path = _orig_compile_bir_kernel(bir_json, tmpdir, neff_name)
    try:
        _os.makedirs(_NEFF_CACHE_DIR, exist_ok=True)
        tmp = cpath + ".tmp"
        _shutil.copy(neff_path, tmp)
        _os.replace(tmp, cpath)
    except Exception:
        pass
    return neff_path


_bass2jax.compile_bir_kernel = _cached_compile_bir_kernel
_bass_utils_mod.compile_bir_kernel = _cached_compile_bir_kernel
# ---------------------------------------------------------------------------

NCORES = 8
B, F, H, O, E, N_NODES = 2048, 512, 256, 1, 128, 4096
BL = B // NCORES          # samples per core = 256
NBT = BL // 128           # b-tiles per core = 2
EP = E // 2               # e-pairs = 64
KI = F // 128             # contraction chunks over input features = 4
W2N = 130                 # [delta1 | root1 | zero-pad] free dim (even)

F32 = mybir.dt.float32
BF16 = mybir.dt.bfloat16
MULT = mybir.AluOpType.mult
ADD = mybir.AluOpType.add
RELU = mybir.ActivationFunctionType.Relu
COPY = mybir.ActivationFunctionType.Copy

_CACHE = {}


def _build_nc():
    nc = bass.Bass()
    xt_d = nc.dram_tensor("xt", (F, BL), BF16, kind="ExternalInput")
    dl_d = nc.dram_tensor("dl", (EP, F, 2 * H), BF16, kind="ExternalInput")
    r0_d = nc.dram_tensor("r0", (F, H), BF16, kind="ExternalInput")
    cols_d = nc.dram_tensor("cols", (BL, E), F32, kind="ExternalInput")
    colse_d = nc.dram_tensor("colse", (E, BL), BF16, kind="ExternalInput")
    d1t_d = nc.dram_tensor("d1t", (E, H), BF16, kind="ExternalInput")
    r1t_d = nc.dram_tensor("r1t", (1, H), BF16, kind="ExternalInput")
    out_d = nc.dram_tensor("out", (BL, 1), F32, kind="ExternalOutput")

    with TileContext(nc) as tc:
        with (
            tc.tile_pool(name="const", bufs=1) as cpool,
            tc.tile_pool(name="acc", bufs=NBT) as apool,
            tc.tile_pool(name="dl", bufs=6) as dpool,
            tc.tile_pool(name="psum", bufs=6, space="PSUM") as ppool,
            tc.tile_pool(name="psum_s", bufs=2, space="PSUM") as pspool,
            tc.tile_pool(name="stage", bufs=6) as spool,
            tc.tile_pool(name="sc", bufs=4) as scpool,
            tc.tile_pool(name="misc", bufs=8) as mpool,
        ):
            # --- resident loads -------------------------------------------
            xt_sb = cpool.tile([128, KI * BL], BF16, tag="xt")
            nc.sync.dma_start(
                xt_sb[:].rearrange("p (k b) -> p k b", k=KI),
                xt_d[:].rearrange("(k p) b -> p k b", p=128),
            )
            r0_sb = cpool.tile([128, KI * H], BF16, tag="r0")
            nc.sync.dma_start(
                r0_sb[:].rearrange("p (k o) -> p k o", k=KI),
                r0_d[:].rearrange("(k p) o -> p k o", p=128),
            )
            cols_sb = cpool.tile([128, NBT * E], F32, tag="cols")
            nc.sync.dma_start(
                cols_sb[:].rearrange("p (t n) -> p t n", t=NBT),
                cols_d[:].rearrange("(t p) n -> p t n", p=128),
            )
            colse_sb = cpool.tile([128, BL], BF16, tag="colse")
            nc.sync.dma_start(colse_sb[:], colse_d[:])
            d1t_sb = cpool.tile([128, H], BF16, tag="d1t")
            nc.sync.dma_start(d1t_sb[:], d1t_d[:])
            r1t_sb = cpool.tile([128, H], BF16, tag="r1t")
            nc.sync.dma_start(r1t_sb[:1, :], r1t_d[:])
            ones_sb = cpool.tile([128, 128], BF16, tag="ones")
            nc.gpsimd.memset(ones_sb[:1, :], 1.0)

            def x_lhsT(k, bt):
                # stationary operand: x^T chunk [128 (i), 128 (b)]
                return xt_sb[:, k * BL + bt * 128 : k * BL + bt * 128 + 128]

            # --- layer-2 per-sample weights --------------------------------
            # wt[b,h] = sum_e cols[b,e]*delta1[0,h,e] + root1[h]
            #         = colsE^T @ delta1[0]^T  (K=E=128)  +  ones^T @ root1^T
            # Emitted a few e-pairs into stage 1 (PE executes in program
            # order) so these small matmuls run at warm clock and overlap.
            wts = []

            def emit_wts():
                for bt in range(NBT):
                    psw = pspool.tile([128, H], F32, tag="ps_s")
                    nc.tensor.matmul(
                        psw[:],
                        colse_sb[:, bt * 128 : (bt + 1) * 128],
                        d1t_sb[:],
                        start=True,
                        stop=False,
                    )
                    nc.tensor.matmul(
                        psw[:], ones_sb[:1, :], r1t_sb[:1, :], start=False, stop=True
                    )
                    wt = cpool.tile([128, H], F32, tag=f"wt{bt}")
                    nc.scalar.activation(wt[:], psw[:], COPY)
                    wts.append(wt)

            # --- acc init: acc[bt] = x @ root0 ----------------------------
            accs = []
            accg = []
            for bt in range(NBT):
                ps = pspool.tile([128, H], F32, tag="ps_s")
                for k in range(KI):
                    nc.tensor.matmul(
                        ps[:],
                        x_lhsT(k, bt),
                        r0_sb[:, k * H : (k + 1) * H],
                        start=(k == 0),
                        stop=(k == KI - 1),
                    )
                acc = apool.tile([128, H], F32, tag="acc")
                nc.scalar.activation(acc[:], ps[:], COPY)
                accs.append(acc)
                # second accumulator for the GPSIMD-routed edge slots
                ag = apool.tile([128, H], F32, tag="accg")
                nc.gpsimd.memset(ag[:], 0.0)
                accg.append(ag)

            # --- stage 1: stream delta, accumulate scaled matmuls ---------
            # PE: ps = x @ [W_{2ep} | W_{2ep+1}] ; ACT: evacuate PSUM->SBUF;
            # DVE: acc = stage_half * colsT[:, e] + acc (all-SBUF fused op)
            for ep in range(EP):
                dlt = dpool.tile([128, KI * 2 * H], BF16, tag="dl")
                nc.sync.dma_start(
                    dlt[:].rearrange("p (k n) -> p k n", k=KI),
                    dl_d[ep].rearrange("(k p) n -> p k n", p=128),
                )
                for bt in range(NBT):
                    ps = ppool.tile([128, 2 * H], F32, tag="ps")
                    for k in range(KI):
                        nc.tensor.matmul(
                            ps[:],
                            x_lhsT(k, bt),
                            dlt[:, k * 2 * H : (k + 1) * 2 * H],
                            start=(k == 0),
                            stop=(k == KI - 1),
                        )
                    stage = spool.tile([128, 2 * H], F32, tag="stage")
                    nc.scalar.activation(stage[:], ps[:], COPY)
                    for half in range(2):
                        e = 2 * ep + half
                        half_ap = stage[:, half * H : (half + 1) * H]
                        col_ap = cols_sb[:, bt * E + e : bt * E + e + 1]
                        if e % 3 == 2:
                            # route every 3rd edge slot via a cheap DVE scaled
                            # copy (tensor_scalar runs in 2x mode, under the
                            # DRAIN threshold) + a GPSIMD add into a second
                            # accumulator, shortening the serial DVE STT chain
                            sc = scpool.tile([128, H], F32, tag="sc")
                            nc.vector.tensor_scalar(
                                sc[:], half_ap, col_ap, None, MULT
                            )
                            nc.gpsimd.tensor_add(accg[bt][:], sc[:], accg[bt][:])
                        else:
                            nc.vector.scalar_tensor_tensor(
                                out=accs[bt][:],
                                in0=half_ap,
                                scalar=col_ap,
                                in1=accs[bt][:],
                                op0=MULT,
                                op1=ADD,
                            )
                if ep == 2:
                    emit_wts()

            # --- layer 2: out[b] = relu(sum_h relu(acc)[b,h] * wt[b,h]) ----
            for bt in range(NBT):
                # merge the GPSIMD accumulator, then relu
                nc.vector.tensor_add(accs[bt][:], accg[bt][:], accs[bt][:])
                h1 = mpool.tile([128, H], F32, tag="h1")
                nc.scalar.activation(h1[:], accs[bt][:], RELU)
                junk = mpool.tile([128, H], F32, tag="junk")
                res = mpool.tile([128, 1], F32, tag="res")
                nc.vector.scalar_tensor_tensor(
                    out=junk[:],
                    in0=h1[:],
                    scalar=1.0,
                    in1=wts[bt][:],
                    op0=MULT,
                    op1=MULT,
                    accum_out=res[:],
                )
                resr = mpool.tile([128, 1], F32, tag="resr")
                nc.scalar.activation(resr[:], res[:], RELU)
                nc.sync.dma_start(
                    out_d[:].rearrange("(t p) o -> t p o", p=128)[bt], resr[:]
                )
    _legalize_wait_counts(nc)
    return nc


def _prep_inputs(x, node_idx, path_mat, root0, root1, delta0, delta1):
    bf16 = ml_dtypes.bfloat16
    x = np.asarray(x, np.float32)
    path_mat = np.asarray(path_mat, np.float32)
    root0 = np.asarray(root0, np.float32)
    root1 = np.asarray(root1, np.float32)
    delta0 = np.asarray(delta0, np.float32)
    delta1 = np.asarray(delta1, np.float32)
    colsT = path_mat.T[np.asarray(node_idx, dtype=np.int64)].astype(np.float32)
    # delta0 (H, F, E) -> (E, F, H) -> pair e's along the free axis
    dt_ = np.ascontiguousarray(delta0.transpose(2, 1, 0))
    dl = np.ascontiguousarray(
        dt_.reshape(EP, 2, F, H).transpose(0, 2, 1, 3)
    ).reshape(EP, F, 2 * H)
    dl16 = dl.astype(bf16)
    xT16 = np.ascontiguousarray(x.T).astype(bf16)  # (F, B)
    r016 = root0.astype(bf16)
    colsE16 = np.ascontiguousarray(colsT.T).astype(bf16)  # (E, B)
    d1t = np.ascontiguousarray(delta1[0].T).astype(bf16)  # (E, H)
    r1t = np.ascontiguousarray(root1.T).astype(bf16)  # (1, H)

    in_maps = []
    for c in range(NCORES):
        sl = slice(c * BL, (c + 1) * BL)
        in_maps.append(
            {
                "xt": np.ascontiguousarray(xT16[:, sl]),
                "dl": dl16,
                "r0": r016,
                "cols": np.ascontiguousarray(colsT[sl]),
                "colse": np.ascontiguousarray(colsE16[:, sl]),
                "d1t": d1t,
                "r1t": r1t,
            }
        )
    return in_maps


def _run(inputs, trace=False, **kw):
    if "nc" not in _CACHE:
        _CACHE["nc"] = _build_nc()
    nc = _CACHE["nc"]
    in_maps = _prep_inputs(**inputs)
    res = run_bass_kernel_spmd(
        nc, in_maps, core_ids=list(range(NCORES)), trace=trace, **kw
    )
    out = np.concatenate([r["out"][:, 0] for r in res.results]).astype(np.float32)
    return out, res


def kernel(**inputs) -> np.ndarray:
    out, _ = _run(inputs)
    return out
